# revision 34
# baseline (speedup 1.0000x reference)
"""Gemma2 fused attention (B=1, S=4096, HID=2304, NH=8, NKV=4, HD=256,
sliding window 2048, softcap 50) on 8 Trainium2 NeuronCores.

Sharding: one query head per core (its GQA kv head recomputed per core);
o_proj is sharded over the contraction dim, per-core partials are summed
on the host.

Per-core math (core c, head h=c, kv group g=c//2):
  qT,kT = (W @ X.T) in [head_dim, tok] layout, RoPE'd on device (cos/sin
  tables precomputed on host; attention scale folded into Wq exactly).
  v in [tok, head_dim] layout.
  S.T[k,q] = kT.T @ qT; u = tanh(S.T/50); E = exp(50*u + mask) in bf16
  (softcap bounds logits to +-50 so no max-subtraction is needed).
  Mask handled per 128(k) x 512(q) block: all-zero blocks skip the add,
  fully-masked blocks are skipped entirely, mixed blocks add mask*0.02
  from a host-packed block stack (data-driven, no pattern assumption).
  Z = ones.T @ E (PSUM row), attnT = (E @ v).T via lhsT=v chunks.
  out_partial[tok, 2304] = attnT.T @ WoT with 1/Z fused into the
  PSUM->SBUF copy. Host sums the 8 partials.
"""

import numpy as np
import ml_dtypes
from contextlib import ExitStack

import concourse.bass as bass
import concourse.tile as tile
import concourse.mybir as mybir
from concourse.bass_utils import run_bass_kernel_spmd
from concourse.vector_clock import ScopedClock

N_CORES = 8
HID = 2304
NH, NKV, HD = 8, 4, 256
SCALE = 256.0 ** -0.5
SOFTCAP = 50.0
ROPE_THETA = 10000.0
KC = HID // 128  # 18 contraction chunks for the projections

BF16 = mybir.dt.bfloat16
F32 = mybir.dt.float32
AF = mybir.ActivationFunctionType

TRACE = False  # test harness flips this to get NTFF exec time


class TC(tile.TileContext):
    """TileContext whose final drain splits sem waits one-per-instruction
    (this walrus rejects instructions carrying more than one wait)."""

    def _drain_and_barrier(self, tick_clock, wait_clock):
        probe = self.nc.sync.nop(nofuse=True, hint="drain_waits")
        wait_clock.add_sem_waits(
            probe.ins, ScopedClock({None: tick_clock.global_clock})
        )
        waits = list(probe.ins.sync_info.on_wait)
        probe.ins.sync_info.on_wait = waits[:1]
        rest = waits[1:]
        while rest:
            extra = self.nc.sync.nop(nofuse=True, hint="drain_waits")
            extra.ins.sync_info = mybir.SyncInfo(on_wait=rest[:1], on_update=[])
            rest = rest[1:]
        self.nc.sync.drain()
        self.nc.all_engine_barrier()
        popped = self.nc._tile_sem_poison_stack.pop()
        assert popped is self._sem_poison
        self.nc.clear_and_free_semaphores(list(self.sems.allocated().values()))
        self.nc.all_engine_barrier()


def split_multi_waits(nc):
    """Split multi-wait instructions: extras move onto same-engine NoOps
    inserted immediately before (engines execute in program order)."""
    ctr = 0
    for f in nc.m.functions:
        for b in f.blocks:
            insts = list(b.instructions)
            new = []
            changed = False
            for inst in insts:
                si = inst.sync_info
                if si is not None and len(si.on_wait) > 1:
                    waits = list(si.on_wait)
                    for w in waits[:-1]:
                        ctr += 1
                        nop = mybir.InstNoOp(
                            name=f"I-waitsplit-{ctr}",
                            engine=inst.engine,
                            debug=inst.debug,
                            sync_info=mybir.SyncInfo(on_wait=[w], on_update=[]),
                        )
                        new.append(nop)
                    inst.sync_info = mybir.SyncInfo(
                        on_wait=[waits[-1]], on_update=list(si.on_update)
                    )
                    changed = True
                new.append(inst)
            if changed:
                b.instructions = new
    return ctr


def _classify_mask(mask, S):
    """Per (k-chunk 128, q-block 512) block: 'skip' (fully masked),
    'clean' (all zero) or mixed (apply additively). Mixed blocks are
    deduplicated by content (causal/sliding-window masks repeat a handful
    of boundary patterns) so the whole stack stays resident in SBUF.
    Returns plan and the unique-block stack scaled by 1/SOFTCAP."""
    maskT = np.ascontiguousarray(np.asarray(mask, np.float32)[0, 0].T)  # [k, q]
    nj, nq = S // 128, S // 512
    blocks = maskT.reshape(nj, 128, nq, 512)
    mx = blocks.max(axis=(1, 3))
    mn = blocks.min(axis=(1, 3))
    skip = mx < -1e8
    clean = (mx == 0.0) & (mn == 0.0)
    plan = []
    mix_blocks = []
    uniq = {}

    def add_block(j, qb):
        """Returns (uniq_idx, q0, q1): dedup index + active column range
        (columns outside [q0,q1) are fully masked and skipped)."""
        raw = maskT[j * 128:(j + 1) * 128, qb * 512:(qb + 1) * 512]
        blk = (raw * (1.0 / SOFTCAP)).astype(ml_dtypes.bfloat16)
        key = blk.tobytes()
        if key not in uniq:
            uniq[key] = len(mix_blocks)
            mix_blocks.append(blk)
        act = np.where((raw > -1e8).any(axis=0))[0]
        q0, q1 = (int(act.min()), int(act.max()) + 1) if act.size else (0, 512)
        return uniq[key], q0, q1

    for qb in range(nq):
        row = []
        for j in range(nj):
            if skip[j, qb]:
                continue
            if clean[j, qb]:
                row.append((j, -1, 0, 512))
            else:
                row.append((j,) + add_block(j, qb))
        if not row:
            # fully-masked q-block (unreachable for causal masks): keep the
            # diagonal chunks so the PSUM accumulations are still defined
            for j in range(4 * qb, 4 * qb + 4):
                row.append((j,) + add_block(j, qb))
        # a full-width chunk must come first: it carries start=True for the
        # PSUM accumulation and the full-width zacc init
        row.sort(key=lambda e: 0 if e[3] - e[2] == 512 else 1)
        plan.append(row)
    if mix_blocks:
        maskb = np.stack(mix_blocks)  # [n, 128, 512] bf16
    else:
        maskb = np.zeros((1, 128, 512), ml_dtypes.bfloat16)
    return plan, maskb


def _build(S, plan, nmix):
    """Emit the SPMD program (identical for all cores; only data differs).

    All DRAM inputs are host-prepacked into exact SBUF images
    ([128 partitions, X] with long contiguous per-partition rows) so each
    tensor loads with a couple of large full-rate DMAs instead of dozens
    of 1KB-row transfers (each dma_start costs ~650ns of Sync-engine
    descriptor generation)."""
    NT = S // 512  # token/query 512-blocks
    nc = bass.Bass("TRN2", target_bir_lowering=False, debug=False,
                   num_devices=N_CORES)

    xt_d = nc.dram_tensor("xt", [NT, 128, KC * 512], BF16,
                          kind="ExternalInput")
    wqk_d = nc.dram_tensor("wqk", [128, KC * 512], BF16,
                           kind="ExternalInput")
    wv_d = nc.dram_tensor("wv", [128, KC * 256], BF16, kind="ExternalInput")
    wo_d = nc.dram_tensor("wo", [128, 2 * HID], BF16, kind="ExternalInput")
    cos_d = nc.dram_tensor("cost", [128, S], F32, kind="ExternalInput")
    sin_d = nc.dram_tensor("sint", [128, S], F32, kind="ExternalInput")
    maskb_d = nc.dram_tensor("maskb", [128, nmix * 512], BF16,
                             kind="ExternalInput")
    out_d = nc.dram_tensor("out", [S, HID], BF16, kind="ExternalOutput")

    with ExitStack() as ctx:
        tc = ctx.enter_context(TC(nc))
        P = lambda name, bufs, space="SBUF": ctx.enter_context(
            tc.tile_pool(name=name, bufs=bufs, space=space))

        wpool = P("w", 1)
        xpool = P("x", 2)
        cspool = P("cs", 2)
        qkpool = P("qk", 1)
        vpool = P("v", 1)
        tmppool = P("tmp", 4)
        upool = P("u", 3)
        epool = P("e", 4)
        apool = P("a", 1)
        zpool = P("z", 2)
        zapool = P("za", 2)
        opool = P("o", 3)
        rpool = P("r", 1)

        ps_qk = P("ps_qk", 1, "PSUM")
        ps_v = P("ps_v", 2, "PSUM")
        ps_pair = P("ps_pair", 1, "PSUM")
        ps_o = P("ps_o", 2, "PSUM")

        # --- resident weights / constants; wo + masks are DMA'd from
        # inside phase A block 0 AFTER the critical wqk/xt transfers so
        # they don't steal startup HBM bandwidth ---
        wqk = wpool.tile([128, KC * 512], BF16, tag="wqk")
        wv = wpool.tile([128, KC * 256], BF16, tag="wv")
        wo = wpool.tile([128, 2 * HID], BF16, tag="wo")
        masks = wpool.tile([128, nmix * 512], BF16, tag="masks")
        ones = wpool.tile([128, 1], BF16, tag="ones")
        nc.vector.memset(ones[:], 1.0)
        ones1 = wpool.tile([1, 1], F32, tag="ones1")
        nc.vector.memset(ones1[:], 1.0)

        # persistent activations (bf16, [128, S] each)
        qlo = qkpool.tile([128, S], BF16, tag="qlo")
        qhi = qkpool.tile([128, S], BF16, tag="qhi")
        klo = qkpool.tile([128, S], BF16, tag="klo")
        khi = qkpool.tile([128, S], BF16, tag="khi")
        vt = vpool.tile([128, (S // 128) * 256], BF16, tag="vt")
        alo = apool.tile([128, S], BF16, tag="alo")
        ahi = apool.tile([128, S], BF16, tag="ahi")
        rc = rpool.tile([128, S // 128], F32, tag="rc")

        qk_dst = [qlo, qhi, klo, khi]

        def phase_a(T):
            """Return emission units (closures) for QKV block T."""
            c0 = T * 512
            xt = xpool.tile([128, KC * 512], BF16, tag="xt")
            units = []

            def dma_unit():
                H = KC * 512 // 2
                if T == 0:
                    # growing interleaved pieces: the first (1-chunk) piece
                    # lands ~1.5us after issue so real matmuls start early;
                    # later pieces are big for full DMA rate. masks/wo/wv
                    # ride behind the critical transfers.
                    edges = [0, 1, 3, 6, 9, 12, 15, 18]
                    for g in range(len(edges) - 1):
                        a, b = edges[g] * 512, edges[g + 1] * 512
                        nc.sync.dma_start(wqk[:, a:b], wqk_d[:, a:b])
                        nc.sync.dma_start(xt[:, a:b], xt_d[T, :, a:b])
                    # non-critical loads ride on the SAME queue so they
                    # stay strictly behind the wqk/xt pieces above
                    nc.sync.dma_start(wv[:], wv_d[:, :])
                    nc.sync.dma_start(masks[:], maskb_d[:, :])
                    nc.sync.dma_start(wo[:], wo_d[:, :])
                else:
                    nc.sync.dma_start(xt[:, :H], xt_d[T, :, :H])
                    nc.sync.dma_start(xt[:, H:], xt_d[T, :, H:])
            units.append(dma_unit)

            cos = cspool.tile([128, 512], F32, tag="cos")
            sin = cspool.tile([128, 512], F32, tag="sin")

            def cs_unit():
                nc.sync.dma_start(cos[:], cos_d[:, c0:c0 + 512])
                nc.sync.dma_start(sin[:], sin_d[:, c0:c0 + 512])
            units.append(cs_unit)

            def qk_pair(p):
                qp = ps_qk.tile([128, 1024], F32, tag="ps_qk")
                for h in range(2):
                    ft = 2 * p + h
                    for kc in range(KC):
                        nc.tensor.matmul(
                            qp[:, h * 512:(h + 1) * 512],
                            wqk[:, kc * 512 + ft * 128:
                                kc * 512 + ft * 128 + 128],
                            xt[:, kc * 512:(kc + 1) * 512],
                            start=(kc == 0), stop=(kc == KC - 1))
                plo, phi = qp[:, 0:512], qp[:, 512:1024]
                dlo, dhi = qk_dst[2 * p], qk_dst[2 * p + 1]
                t1 = tmppool.tile([128, 512], F32, tag="tmp")
                nc.vector.tensor_mul(t1[:], phi, sin[:])
                t2 = tmppool.tile([128, 512], F32, tag="tmp")
                nc.vector.tensor_mul(t2[:], plo, cos[:])
                nc.vector.tensor_sub(dlo[:, c0:c0 + 512], t2[:], t1[:])
                t3 = tmppool.tile([128, 512], F32, tag="tmp")
                nc.vector.tensor_mul(t3[:], plo, sin[:])
                t4 = tmppool.tile([128, 512], F32, tag="tmp")
                nc.vector.tensor_mul(t4[:], phi, cos[:])
                nc.vector.tensor_add(dhi[:, c0:c0 + 512], t4[:], t3[:])

            def qk_chunk_major():
                # block 0 is paced by the weight/activation DMAs: keep 4
                # accumulations in flight (borrowing the idle B-phase pair
                # bank) so each arriving chunk feeds 4 matmuls
                qpA = ps_qk.tile([128, 1024], F32, tag="ps_qk")
                pspA = ps_pair.tile([128, 1024], F32, tag="ps_pair")
                psA = [qpA[:, 0:512], qpA[:, 512:1024],
                       pspA[:, 0:512], pspA[:, 512:1024]]
                for kc in range(KC):
                    for ft in range(4):
                        nc.tensor.matmul(
                            psA[ft],
                            wqk[:, kc * 512 + ft * 128: kc * 512 + ft * 128 + 128],
                            xt[:, kc * 512:(kc + 1) * 512],
                            start=(kc == 0), stop=(kc == KC - 1))
                for pair in range(2):
                    plo, phi = psA[2 * pair], psA[2 * pair + 1]
                    dlo, dhi = qk_dst[2 * pair], qk_dst[2 * pair + 1]
                    t1 = tmppool.tile([128, 512], F32, tag="tmp")
                    nc.vector.tensor_mul(t1[:], phi, sin[:])
                    t2 = tmppool.tile([128, 512], F32, tag="tmp")
                    nc.vector.tensor_mul(t2[:], plo, cos[:])
                    nc.vector.tensor_sub(dlo[:, c0:c0 + 512], t2[:], t1[:])
                    t3 = tmppool.tile([128, 512], F32, tag="tmp")
                    nc.vector.tensor_mul(t3[:], plo, sin[:])
                    t4 = tmppool.tile([128, 512], F32, tag="tmp")
                    nc.vector.tensor_mul(t4[:], phi, cos[:])
                    nc.vector.tensor_add(dhi[:, c0:c0 + 512], t4[:], t3[:])

            if T == 0:
                units.append(qk_chunk_major)
            else:
                for p in range(2):
                    units.append(lambda p=p: qk_pair(p))

            def v_unit(half):
                ps = ps_v.tile([128, 512], F32, tag="ps_v")
                for sub in range(2):
                    st = half * 2 + sub
                    o = ps[:, sub * 256:(sub + 1) * 256]
                    for kc in range(KC):
                        nc.tensor.matmul(
                            o,
                            xt[:, kc * 512 + st * 128: kc * 512 + st * 128 + 128],
                            wv[:, kc * 256:(kc + 1) * 256],
                            start=(kc == 0), stop=(kc == KC - 1))
                for sub in range(2):
                    st = half * 2 + sub
                    tok = T * 4 + st
                    # ACT is idle during phase A; keeping this off DVE also
                    # stops the v evacuation queueing behind RoPE ops
                    nc.scalar.copy(vt[:, tok * 256:(tok + 1) * 256],
                                   ps[:, sub * 256:(sub + 1) * 256])

            for half in range(2):
                units.append(lambda half=half: v_unit(half))
            return units

        def phase_b(qb, alt=False):
            """Return emission units for attention q-block qb, one per
            k-chunk PAIR: two chunks share a 2-bank PSUM tile so full-width
            pairs get a single wide tanh/exp (halves the ACT op count).
            The S matmuls of pair i lead the E-consumers of pair i-1 so the
            ACT chain has a full PE iteration of slack."""
            c0 = qb * 512
            zacc = zapool.tile([128, 512], BF16, tag="za")
            olo = ps_o.tile([128, 512], F32, tag="ps_o")
            ohi = ps_o.tile([128, 512], F32, tag="ps_o")
            row = plan[qb]
            assert len(row) % 2 == 0
            pairs = [(row[2 * i], row[2 * i + 1]) for i in range(len(row) // 2)]
            state = {}

            def s_pair(i):
                pr = pairs[i]
                # in blocks with no woven A work (tail), alternate between
                # the two 2-bank pools for 2-deep pair pipelining
                pool = ps_qk if (alt and i % 2 == 1) else ps_pair
                psp = pool.tile([128, 1024], F32, tag=pool.name, name="psp")
                for half, (j, mix, q0, q1) in enumerate(pr):
                    o = half * 512
                    nc.tensor.matmul(psp[:, o + q0:o + q1],
                                     klo[:, j * 128:(j + 1) * 128],
                                     qlo[:, c0 + q0:c0 + q1],
                                     start=True, stop=False)
                    nc.tensor.matmul(psp[:, o + q0:o + q1],
                                     khi[:, j * 128:(j + 1) * 128],
                                     qhi[:, c0 + q0:c0 + q1],
                                     start=False, stop=True)
                u = upool.tile([128, 1024], F32, tag="u")
                e = epool.tile([128, 1024], BF16, tag="e")
                both_full = all(q1 - q0 == 512 for _, _, q0, q1 in pr)
                if both_full:
                    nc.scalar.activation(u[:], psp[:], AF.Tanh,
                                         scale=1.0 / SOFTCAP)
                else:
                    for half, (j, mix, q0, q1) in enumerate(pr):
                        o = half * 512
                        nc.scalar.activation(u[:, o + q0:o + q1],
                                             psp[:, o + q0:o + q1], AF.Tanh,
                                             scale=1.0 / SOFTCAP)
                any_mask = any(mix >= 0 for _, mix, _, _ in pr)
                u2 = None
                if any_mask:
                    u2 = upool.tile([128, 1024], F32, tag="u", name="u2")
                srcs = []
                for half, (j, mix, q0, q1) in enumerate(pr):
                    o = half * 512
                    if mix >= 0:
                        nc.vector.tensor_add(u2[:, o + q0:o + q1],
                                             u[:, o + q0:o + q1],
                                             masks[:, mix * 512 + q0:
                                                   mix * 512 + q1])
                        srcs.append(u2)
                    else:
                        srcs.append(u)
                if both_full and srcs[0] is srcs[1]:
                    nc.scalar.activation(e[:], srcs[0][:], AF.Exp,
                                         scale=SOFTCAP)
                else:
                    for half, (j, mix, q0, q1) in enumerate(pr):
                        o = half * 512
                        nc.scalar.activation(e[:, o + q0:o + q1],
                                             srcs[half][:, o + q0:o + q1],
                                             AF.Exp, scale=SOFTCAP)
                # bf16 accumulate: 2x DVE rate; per-partition rounding
                # errors average out in the final cross-partition sum
                for half, (j, mix, q0, q1) in enumerate(pr):
                    o = half * 512
                    if i == 0 and half == 0:
                        nc.vector.tensor_copy(zacc[:], e[:, 0:512])
                    else:
                        nc.vector.tensor_add(zacc[:, q0:q1], zacc[:, q0:q1],
                                             e[:, o + q0:o + q1])
                state[i] = e

            def mm_pair(i):
                pr = pairs[i]
                e = state.pop(i)
                for half, (j, mix, q0, q1) in enumerate(pr):
                    o = half * 512
                    first = i == 0 and half == 0
                    last = i == len(pairs) - 1 and half == 1
                    nc.tensor.matmul(olo[:, q0:q1],
                                     vt[:, j * 256:j * 256 + 128],
                                     e[:, o + q0:o + q1],
                                     start=first, stop=last)
                    nc.tensor.matmul(ohi[:, q0:q1],
                                     vt[:, j * 256 + 128:(j + 1) * 256],
                                     e[:, o + q0:o + q1],
                                     start=first, stop=last)

            def tail_unit():
                nc.vector.tensor_copy(alo[:, c0:c0 + 512], olo[:])
                nc.vector.tensor_copy(ahi[:, c0:c0 + 512], ohi[:])
                zps = ps_v.tile([1, 512], F32, tag="ps_v")
                nc.tensor.matmul(zps[:], ones[:], zacc[:],
                                 start=True, stop=True)
                zrow = zpool.tile([1, 512], F32, tag="z")
                nc.vector.tensor_copy(zrow[:], zps[:])
                # transpose Z [1,512] -> [128,4] with 4 tiny PE matmuls
                # (zrow slice as stationary, 1x1 ones as moving operand);
                # ~0.5us on-chip vs the ~5us DRAM round-trip it replaces
                zcps = ps_v.tile([128, 512], F32, tag="ps_v")
                for b in range(4):
                    nc.tensor.matmul(zcps[:, b:b + 1],
                                     zrow[0:1, b * 128:(b + 1) * 128],
                                     ones1[:], start=True, stop=True)
                nc.vector.reciprocal(rc[:, 4 * qb:4 * qb + 4],
                                     zcps[:, 0:4])

            units = [lambda: s_pair(0)]
            for i in range(1, len(pairs)):
                units.append(lambda i=i: (s_pair(i), mm_pair(i - 1)))
            units.append(lambda: (mm_pair(len(pairs) - 1), tail_unit()))
            return units

        # PE warmup: a few throwaway matmuls so HAM reaches 8/8 before
        # the first real accumulation
        scratch = wpool.tile([128, 512], BF16, tag="scratch")
        nc.vector.memset(scratch[:], 0.0)
        wps = ps_pair.tile([128, 1024], F32, tag="ps_pair")
        for _ in range(10):
            nc.tensor.matmul(wps[:, :512], scratch[:, :128], scratch[:],
                             start=True, stop=True)

        # output projection units (one per (tok-tile, feat-block)); the
        # 1/Z normalization is fused into the PSUM->SBUF copy. The five
        # feat-blocks of a tok-tile stage into one [128, 2304] bf16 tile
        # that leaves as a single full-rate DMA on the gpsimd queue.
        fbs = [(0, 512), (512, 512), (1024, 512), (1536, 512), (2048, 256)]
        osb_state = {}

        def proj_unit(t, fi, act_heavy):
            f0, fw = fbs[fi]
            ps = ps_v.tile([128, 512], F32, tag="ps_v")
            nc.tensor.matmul(ps[:, :fw], alo[:, t * 128:(t + 1) * 128],
                             wo[:, f0:f0 + fw], start=True, stop=False)
            nc.tensor.matmul(ps[:, :fw], ahi[:, t * 128:(t + 1) * 128],
                             wo[:, HID + f0:HID + f0 + fw],
                             start=False, stop=True)
            if fi == 0:
                osb = opool.tile([128, HID], BF16, tag="o", name=f"osb{t}")
                osb_state[t] = osb
            osb = osb_state[t]
            # early blocks run next to ACT-free phase-A work -> lean ACT;
            # tail blocks run next to ACT-bound B blocks -> lean DVE
            on_act = (fi % 2 == 0) if act_heavy else (fi == 4)
            if on_act:
                nc.scalar.activation(osb[:, f0:f0 + fw], ps[:, :fw], AF.Copy,
                                     scale=rc[:, t:t + 1])
            else:
                nc.vector.tensor_scalar_mul(osb[:, f0:f0 + fw], ps[:, :fw],
                                            rc[:, t:t + 1])
            # two pipelined half-DMAs per tile: the first goes out while
            # the last feat-blocks are still being projected
            if fi == 1:
                nc.gpsimd.dma_start(out_d[t * 128:(t + 1) * 128, :1024],
                                    osb[:, :1024])
            elif fi == len(fbs) - 1:
                osb = osb_state.pop(t)
                nc.gpsimd.dma_start(out_d[t * 128:(t + 1) * 128, 1024:],
                                    osb[:, 1024:])

        def phase_c(qb, tail=False):
            act_heavy = qb < 4 or qb == NT - 1
            t0 = 4 * qb
            if not tail:
                return [lambda t=t, fi=fi: proj_unit(t, fi, act_heavy)
                        for t in range(t0, t0 + 4)
                        for fi in range(len(fbs))]
            # tail ordering: two tiles advance together so PE matmuls stay
            # dense while copies trail on both ACT and DVE
            return [lambda t=t, fi=fi: proj_unit(t, fi, act_heavy)
                    for tp in (t0, t0 + 2)
                    for fi in range(len(fbs))
                    for t in (tp, tp + 1)]

        def weave(bunits, aunits):
            """Alternate A and B units, with each slot's independent A
            units emitted BEFORE the B unit so stalled B consumers never
            block independent A matmuls in the in-order PE queue."""
            out = []
            na, nb = len(aunits), len(bunits)
            ai = 0
            for bi, bu in enumerate(bunits):
                want = (bi + 1) * na // nb
                while ai < want:
                    out.append(aunits[ai])
                    ai += 1
                out.append(bu)
            out.extend(aunits[ai:])
            return out

        for u in phase_a(0):
            u()
        for u in phase_a(1):
            u()
        qorder = list(range(1, NT)) + [0]
        for idx, qb in enumerate(qorder):
            na = idx + 2  # next A block to prefetch
            alt = na >= NT  # no A work left to weave -> tail block
            bunits = phase_b(qb, alt=alt)
            aunits = phase_a(na) if na < NT else []
            if idx >= 1:
                aunits = aunits + phase_c(qorder[idx - 1], tail=alt)
            with nc.named_scope(f"B{qb}"):
                for u in weave(bunits, aunits):
                    u()
        with nc.named_scope("Ctail"):
            for u in phase_c(qorder[-1], tail=True):
                u()


    split_multi_waits(nc)
    return nc


def _sbuf_image(mat, cols):
    """[R, cols] -> [128, (R//128)*cols] SBUF image (chunk-major rows)."""
    R = mat.shape[0]
    return np.ascontiguousarray(
        mat.reshape(R // 128, 128, cols).transpose(1, 0, 2).reshape(
            128, (R // 128) * cols))


def kernel(hidden_states, attention_mask, position_ids, Wqkv, Wo):
    bf16 = ml_dtypes.bfloat16
    hidden = np.asarray(hidden_states, np.float32)
    S = hidden.shape[1]
    NT = S // 512
    X = hidden[0]  # [S, HID]
    XT = np.ascontiguousarray(X.T).astype(bf16)  # [HID, S]
    # per-block SBUF images: [NT, 128, KC*512]
    xt_img = np.ascontiguousarray(
        XT.reshape(KC, 128, NT, 512).transpose(2, 1, 0, 3).reshape(
            NT, 128, KC * 512))

    pos = np.asarray(position_ids)[0].astype(np.float64)
    inv = 1.0 / (ROPE_THETA ** (np.arange(0, HD, 2, dtype=np.float64) / HD))
    freqs = inv[:, None] * pos[None, :]  # [128, S]
    cosT = np.cos(freqs).astype(np.float32)
    sinT = np.sin(freqs).astype(np.float32)

    plan, maskb = _classify_mask(attention_mask, S)
    nmix = maskb.shape[0]
    maskb_img = np.ascontiguousarray(
        maskb.transpose(1, 0, 2).reshape(128, nmix * 512))

    Wqkv = np.asarray(Wqkv, np.float32)
    Wo = np.asarray(Wo, np.float32)
    ones = np.ones((128, 1), bf16)

    in_maps = []
    for c in range(N_CORES):
        g = c // (NH // NKV)
        wq = Wqkv[c * HD:(c + 1) * HD] * SCALE  # exact: SCALE = 2**-4
        wk = Wqkv[NH * HD + g * HD: NH * HD + (g + 1) * HD]
        wv = Wqkv[(NH + NKV) * HD + g * HD: (NH + NKV) * HD + (g + 1) * HD]
        wqk = np.ascontiguousarray(
            np.concatenate([wq.T, wk.T], axis=1)).astype(bf16)  # [HID, 512]
        wvt = np.ascontiguousarray(wv.T).astype(bf16)           # [HID, 256]
        wot = np.ascontiguousarray(Wo[:, c * HD:(c + 1) * HD].T).astype(bf16)
        in_maps.append({
            "xt": xt_img,
            "wqk": _sbuf_image(wqk, 512),
            "wv": _sbuf_image(wvt, 256),
            "wo": _sbuf_image(wot, HID),
            "cost": cosT, "sint": sinT, "maskb": maskb_img,
        })

    nc = _build(S, plan, nmix)
    res = run_bass_kernel_spmd(nc, in_maps, list(range(N_CORES)),
                               trace=TRACE)
    out = res.results[0]["out"].astype(np.float64)
    for c in range(1, N_CORES):
        out += res.results[c]["out"].astype(np.float64)
    kernel.last_exec_time_ns = res.exec_time_ns
    kernel.last_results = res
    return out[None].astype(np.float32)


kernel.last_exec_time_ns = None
kernel.last_results = None



# revision 37
# speedup vs baseline: 1.0033x; 1.0033x over previous
"""Gemma2 fused attention (B=1, S=4096, HID=2304, NH=8, NKV=4, HD=256,
sliding window 2048, softcap 50) on 8 Trainium2 NeuronCores.

Sharding: one query head per core (its GQA kv head recomputed per core);
o_proj is sharded over the contraction dim, per-core partials are summed
on the host.

Per-core math (core c, head h=c, kv group g=c//2):
  qT,kT = (W @ X.T) in [head_dim, tok] layout, RoPE'd on device (cos/sin
  tables precomputed on host; attention scale folded into Wq exactly).
  v in [tok, head_dim] layout.
  S.T[k,q] = kT.T @ qT; u = tanh(S.T/50); E = exp(50*u + mask) in bf16
  (softcap bounds logits to +-50 so no max-subtraction is needed).
  Mask handled per 128(k) x 512(q) block: all-zero blocks skip the add,
  fully-masked blocks are skipped entirely, mixed blocks add mask*0.02
  from a host-packed block stack (data-driven, no pattern assumption).
  Z = ones.T @ E (PSUM row), attnT = (E @ v).T via lhsT=v chunks.
  out_partial[tok, 2304] = attnT.T @ WoT with 1/Z fused into the
  PSUM->SBUF copy. Host sums the 8 partials.
"""

import numpy as np
import ml_dtypes
from contextlib import ExitStack

import concourse.bass as bass
import concourse.tile as tile
import concourse.mybir as mybir
from concourse.bass_utils import run_bass_kernel_spmd
from concourse.vector_clock import ScopedClock

N_CORES = 8
HID = 2304
NH, NKV, HD = 8, 4, 256
SCALE = 256.0 ** -0.5
SOFTCAP = 50.0
ROPE_THETA = 10000.0
KC = HID // 128  # 18 contraction chunks for the projections

BF16 = mybir.dt.bfloat16
F32 = mybir.dt.float32
AF = mybir.ActivationFunctionType

TRACE = False  # test harness flips this to get NTFF exec time


class TC(tile.TileContext):
    """TileContext whose final drain splits sem waits one-per-instruction
    (this walrus rejects instructions carrying more than one wait)."""

    def _drain_and_barrier(self, tick_clock, wait_clock):
        probe = self.nc.sync.nop(nofuse=True, hint="drain_waits")
        wait_clock.add_sem_waits(
            probe.ins, ScopedClock({None: tick_clock.global_clock})
        )
        waits = list(probe.ins.sync_info.on_wait)
        probe.ins.sync_info.on_wait = waits[:1]
        rest = waits[1:]
        while rest:
            extra = self.nc.sync.nop(nofuse=True, hint="drain_waits")
            extra.ins.sync_info = mybir.SyncInfo(on_wait=rest[:1], on_update=[])
            rest = rest[1:]
        self.nc.sync.drain()
        self.nc.all_engine_barrier()
        popped = self.nc._tile_sem_poison_stack.pop()
        assert popped is self._sem_poison
        self.nc.clear_and_free_semaphores(list(self.sems.allocated().values()))
        self.nc.all_engine_barrier()


def split_multi_waits(nc):
    """Split multi-wait instructions: extras move onto same-engine NoOps
    inserted immediately before (engines execute in program order)."""
    ctr = 0
    for f in nc.m.functions:
        for b in f.blocks:
            insts = list(b.instructions)
            new = []
            changed = False
            for inst in insts:
                si = inst.sync_info
                if si is not None and len(si.on_wait) > 1:
                    waits = list(si.on_wait)
                    for w in waits[:-1]:
                        ctr += 1
                        nop = mybir.InstNoOp(
                            name=f"I-waitsplit-{ctr}",
                            engine=inst.engine,
                            debug=inst.debug,
                            sync_info=mybir.SyncInfo(on_wait=[w], on_update=[]),
                        )
                        new.append(nop)
                    inst.sync_info = mybir.SyncInfo(
                        on_wait=[waits[-1]], on_update=list(si.on_update)
                    )
                    changed = True
                new.append(inst)
            if changed:
                b.instructions = new
    return ctr


def _classify_mask(mask, S):
    """Per (k-chunk 128, q-block 512) block: 'skip' (fully masked),
    'clean' (all zero) or mixed (apply additively). Mixed blocks are
    deduplicated by content (causal/sliding-window masks repeat a handful
    of boundary patterns) so the whole stack stays resident in SBUF.
    Returns plan and the unique-block stack scaled by 1/SOFTCAP."""
    maskT = np.ascontiguousarray(np.asarray(mask, np.float32)[0, 0].T)  # [k, q]
    nj, nq = S // 128, S // 512
    blocks = maskT.reshape(nj, 128, nq, 512)
    mx = blocks.max(axis=(1, 3))
    mn = blocks.min(axis=(1, 3))
    skip = mx < -1e8
    clean = (mx == 0.0) & (mn == 0.0)
    plan = []
    mix_blocks = []
    uniq = {}

    def add_block(j, qb):
        """Returns (uniq_idx, q0, q1): dedup index + active column range
        (columns outside [q0,q1) are fully masked and skipped)."""
        raw = maskT[j * 128:(j + 1) * 128, qb * 512:(qb + 1) * 512]
        blk = (raw * (1.0 / SOFTCAP)).astype(ml_dtypes.bfloat16)
        key = blk.tobytes()
        if key not in uniq:
            uniq[key] = len(mix_blocks)
            mix_blocks.append(blk)
        act = np.where((raw > -1e8).any(axis=0))[0]
        q0, q1 = (int(act.min()), int(act.max()) + 1) if act.size else (0, 512)
        return uniq[key], q0, q1

    for qb in range(nq):
        row = []
        for j in range(nj):
            if skip[j, qb]:
                continue
            if clean[j, qb]:
                row.append((j, -1, 0, 512))
            else:
                row.append((j,) + add_block(j, qb))
        if not row:
            # fully-masked q-block (unreachable for causal masks): keep the
            # diagonal chunks so the PSUM accumulations are still defined
            for j in range(4 * qb, 4 * qb + 4):
                row.append((j,) + add_block(j, qb))
        # a full-width chunk must come first: it carries start=True for the
        # PSUM accumulation and the full-width zacc init
        row.sort(key=lambda e: 0 if e[3] - e[2] == 512 else 1)
        plan.append(row)
    if mix_blocks:
        maskb = np.stack(mix_blocks)  # [n, 128, 512] bf16
    else:
        maskb = np.zeros((1, 128, 512), ml_dtypes.bfloat16)
    return plan, maskb


def _build(S, plan, nmix):
    """Emit the SPMD program (identical for all cores; only data differs).

    All DRAM inputs are host-prepacked into exact SBUF images
    ([128 partitions, X] with long contiguous per-partition rows) so each
    tensor loads with a couple of large full-rate DMAs instead of dozens
    of 1KB-row transfers (each dma_start costs ~650ns of Sync-engine
    descriptor generation)."""
    NT = S // 512  # token/query 512-blocks
    nc = bass.Bass("TRN2", target_bir_lowering=False, debug=False,
                   num_devices=N_CORES)

    xt_d = nc.dram_tensor("xt", [NT, 128, KC * 512], BF16,
                          kind="ExternalInput")
    wqk_d = nc.dram_tensor("wqk", [128, KC * 512], BF16,
                           kind="ExternalInput")
    wv_d = nc.dram_tensor("wv", [128, KC * 256], BF16, kind="ExternalInput")
    wo_d = nc.dram_tensor("wo", [128, 2 * HID], BF16, kind="ExternalInput")
    cos_d = nc.dram_tensor("cost", [128, S], F32, kind="ExternalInput")
    sin_d = nc.dram_tensor("sint", [128, S], F32, kind="ExternalInput")
    maskb_d = nc.dram_tensor("maskb", [128, nmix * 512], BF16,
                             kind="ExternalInput")
    out_d = nc.dram_tensor("out", [S, HID], BF16, kind="ExternalOutput")

    with ExitStack() as ctx:
        tc = ctx.enter_context(TC(nc))
        P = lambda name, bufs, space="SBUF": ctx.enter_context(
            tc.tile_pool(name=name, bufs=bufs, space=space))

        wpool = P("w", 1)
        xpool = P("x", 2)
        cspool = P("cs", 2)
        qkpool = P("qk", 1)
        vpool = P("v", 1)
        tmppool = P("tmp", 4)
        upool = P("u", 3)
        epool = P("e", 4)
        apool = P("a", 1)
        zpool = P("z", 2)
        zapool = P("za", 2)
        opool = P("o", 3)
        rpool = P("r", 1)

        ps_qk = P("ps_qk", 1, "PSUM")
        ps_v = P("ps_v", 2, "PSUM")
        ps_pair = P("ps_pair", 1, "PSUM")
        ps_o = P("ps_o", 2, "PSUM")

        # --- resident weights / constants; wo + masks are DMA'd from
        # inside phase A block 0 AFTER the critical wqk/xt transfers so
        # they don't steal startup HBM bandwidth ---
        wqk = wpool.tile([128, KC * 512], BF16, tag="wqk")
        wv = wpool.tile([128, KC * 256], BF16, tag="wv")
        wo = wpool.tile([128, 2 * HID], BF16, tag="wo")
        masks = wpool.tile([128, nmix * 512], BF16, tag="masks")
        ones = wpool.tile([128, 1], BF16, tag="ones")
        nc.vector.memset(ones[:], 1.0)
        ones1 = wpool.tile([1, 1], F32, tag="ones1")
        nc.vector.memset(ones1[:], 1.0)

        # persistent activations (bf16, [128, S] each)
        qlo = qkpool.tile([128, S], BF16, tag="qlo")
        qhi = qkpool.tile([128, S], BF16, tag="qhi")
        klo = qkpool.tile([128, S], BF16, tag="klo")
        khi = qkpool.tile([128, S], BF16, tag="khi")
        vt = vpool.tile([128, (S // 128) * 256], BF16, tag="vt")
        alo = apool.tile([128, S], BF16, tag="alo")
        ahi = apool.tile([128, S], BF16, tag="ahi")
        rc = rpool.tile([128, S // 128], F32, tag="rc")

        qk_dst = [qlo, qhi, klo, khi]

        def phase_a(T):
            """Return emission units (closures) for QKV block T."""
            c0 = T * 512
            xt = xpool.tile([128, KC * 512], BF16, tag="xt")
            units = []

            def dma_unit():
                H = KC * 512 // 2
                if T == 0:
                    # growing interleaved pieces: the first (1-chunk) piece
                    # lands ~1.5us after issue so real matmuls start early;
                    # later pieces are big for full DMA rate. masks/wo/wv
                    # ride behind the critical transfers.
                    edges = [0, 1, 3, 6, 9, 12, 15, 18]
                    for g in range(len(edges) - 1):
                        a, b = edges[g] * 512, edges[g + 1] * 512
                        nc.sync.dma_start(wqk[:, a:b], wqk_d[:, a:b])
                        nc.sync.dma_start(xt[:, a:b], xt_d[T, :, a:b])
                    # non-critical loads ride on the SAME queue so they
                    # stay strictly behind the wqk/xt pieces above
                    nc.sync.dma_start(wv[:], wv_d[:, :])
                    nc.sync.dma_start(masks[:], maskb_d[:, :])
                    nc.sync.dma_start(wo[:], wo_d[:, :])
                else:
                    nc.sync.dma_start(xt[:, :H], xt_d[T, :, :H])
                    nc.sync.dma_start(xt[:, H:], xt_d[T, :, H:])
            units.append(dma_unit)

            cos = cspool.tile([128, 512], F32, tag="cos")
            sin = cspool.tile([128, 512], F32, tag="sin")

            def cs_unit():
                nc.sync.dma_start(cos[:], cos_d[:, c0:c0 + 512])
                nc.sync.dma_start(sin[:], sin_d[:, c0:c0 + 512])
            units.append(cs_unit)

            def qk_pair(p):
                qp = ps_qk.tile([128, 1024], F32, tag="ps_qk")
                for h in range(2):
                    ft = 2 * p + h
                    for kc in range(KC):
                        nc.tensor.matmul(
                            qp[:, h * 512:(h + 1) * 512],
                            wqk[:, kc * 512 + ft * 128:
                                kc * 512 + ft * 128 + 128],
                            xt[:, kc * 512:(kc + 1) * 512],
                            start=(kc == 0), stop=(kc == KC - 1))
                plo, phi = qp[:, 0:512], qp[:, 512:1024]
                dlo, dhi = qk_dst[2 * p], qk_dst[2 * p + 1]
                t1 = tmppool.tile([128, 512], F32, tag="tmp")
                nc.vector.tensor_mul(t1[:], phi, sin[:])
                t2 = tmppool.tile([128, 512], F32, tag="tmp")
                nc.vector.tensor_mul(t2[:], plo, cos[:])
                nc.vector.tensor_sub(dlo[:, c0:c0 + 512], t2[:], t1[:])
                t3 = tmppool.tile([128, 512], F32, tag="tmp")
                nc.vector.tensor_mul(t3[:], plo, sin[:])
                t4 = tmppool.tile([128, 512], F32, tag="tmp")
                nc.vector.tensor_mul(t4[:], phi, cos[:])
                nc.vector.tensor_add(dhi[:, c0:c0 + 512], t4[:], t3[:])

            def qk_chunk_major():
                # block 0 is paced by the weight/activation DMAs: keep 4
                # accumulations in flight (borrowing the idle B-phase pair
                # bank) so each arriving chunk feeds 4 matmuls
                qpA = ps_qk.tile([128, 1024], F32, tag="ps_qk")
                pspA = ps_pair.tile([128, 1024], F32, tag="ps_pair")
                psA = [qpA[:, 0:512], qpA[:, 512:1024],
                       pspA[:, 0:512], pspA[:, 512:1024]]
                for kc in range(KC):
                    for ft in range(4):
                        nc.tensor.matmul(
                            psA[ft],
                            wqk[:, kc * 512 + ft * 128: kc * 512 + ft * 128 + 128],
                            xt[:, kc * 512:(kc + 1) * 512],
                            start=(kc == 0), stop=(kc == KC - 1))
                for pair in range(2):
                    plo, phi = psA[2 * pair], psA[2 * pair + 1]
                    dlo, dhi = qk_dst[2 * pair], qk_dst[2 * pair + 1]
                    t1 = tmppool.tile([128, 512], F32, tag="tmp")
                    nc.vector.tensor_mul(t1[:], phi, sin[:])
                    t2 = tmppool.tile([128, 512], F32, tag="tmp")
                    nc.vector.tensor_mul(t2[:], plo, cos[:])
                    nc.vector.tensor_sub(dlo[:, c0:c0 + 512], t2[:], t1[:])
                    t3 = tmppool.tile([128, 512], F32, tag="tmp")
                    nc.vector.tensor_mul(t3[:], plo, sin[:])
                    t4 = tmppool.tile([128, 512], F32, tag="tmp")
                    nc.vector.tensor_mul(t4[:], phi, cos[:])
                    nc.vector.tensor_add(dhi[:, c0:c0 + 512], t4[:], t3[:])

            if T == 0:
                units.append(qk_chunk_major)
            else:
                for p in range(2):
                    units.append(lambda p=p: qk_pair(p))

            def v_unit(half):
                ps = ps_v.tile([128, 512], F32, tag="ps_v")
                for sub in range(2):
                    st = half * 2 + sub
                    o = ps[:, sub * 256:(sub + 1) * 256]
                    for kc in range(KC):
                        nc.tensor.matmul(
                            o,
                            xt[:, kc * 512 + st * 128: kc * 512 + st * 128 + 128],
                            wv[:, kc * 256:(kc + 1) * 256],
                            start=(kc == 0), stop=(kc == KC - 1))
                for sub in range(2):
                    st = half * 2 + sub
                    tok = T * 4 + st
                    # ACT is idle during phase A; keeping this off DVE also
                    # stops the v evacuation queueing behind RoPE ops
                    nc.scalar.copy(vt[:, tok * 256:(tok + 1) * 256],
                                   ps[:, sub * 256:(sub + 1) * 256])

            for half in range(2):
                units.append(lambda half=half: v_unit(half))
            return units

        def phase_b(qb, alt=False):
            """Return emission units for attention q-block qb, one per
            k-chunk PAIR: two chunks share a 2-bank PSUM tile so full-width
            pairs get a single wide tanh/exp (halves the ACT op count).
            The S matmuls of pair i lead the E-consumers of pair i-1 so the
            ACT chain has a full PE iteration of slack."""
            c0 = qb * 512
            zacc = zapool.tile([128, 512], BF16, tag="za")
            olo = ps_o.tile([128, 512], F32, tag="ps_o")
            ohi = ps_o.tile([128, 512], F32, tag="ps_o")
            row = plan[qb]
            assert len(row) % 2 == 0
            pairs = [(row[2 * i], row[2 * i + 1]) for i in range(len(row) // 2)]
            state = {}

            def s_pair(i):
                pr = pairs[i]
                # in blocks with no woven A work (tail), alternate between
                # the two 2-bank pools for 2-deep pair pipelining
                pool = ps_qk if (alt and i % 2 == 1) else ps_pair
                psp = pool.tile([128, 1024], F32, tag=pool.name, name="psp")
                for half, (j, mix, q0, q1) in enumerate(pr):
                    o = half * 512
                    nc.tensor.matmul(psp[:, o + q0:o + q1],
                                     klo[:, j * 128:(j + 1) * 128],
                                     qlo[:, c0 + q0:c0 + q1],
                                     start=True, stop=False)
                    nc.tensor.matmul(psp[:, o + q0:o + q1],
                                     khi[:, j * 128:(j + 1) * 128],
                                     qhi[:, c0 + q0:c0 + q1],
                                     start=False, stop=True)
                u = upool.tile([128, 1024], F32, tag="u")
                e = epool.tile([128, 1024], BF16, tag="e")
                both_full = all(q1 - q0 == 512 for _, _, q0, q1 in pr)
                if both_full:
                    nc.scalar.activation(u[:], psp[:], AF.Tanh,
                                         scale=1.0 / SOFTCAP)
                else:
                    for half, (j, mix, q0, q1) in enumerate(pr):
                        o = half * 512
                        nc.scalar.activation(u[:, o + q0:o + q1],
                                             psp[:, o + q0:o + q1], AF.Tanh,
                                             scale=1.0 / SOFTCAP)
                any_mask = any(mix >= 0 for _, mix, _, _ in pr)
                u2 = None
                if any_mask:
                    u2 = upool.tile([128, 1024], F32, tag="u", name="u2")
                srcs = []
                for half, (j, mix, q0, q1) in enumerate(pr):
                    o = half * 512
                    if mix >= 0:
                        nc.vector.tensor_add(u2[:, o + q0:o + q1],
                                             u[:, o + q0:o + q1],
                                             masks[:, mix * 512 + q0:
                                                   mix * 512 + q1])
                        srcs.append(u2)
                    else:
                        srcs.append(u)
                if both_full and srcs[0] is srcs[1]:
                    nc.scalar.activation(e[:], srcs[0][:], AF.Exp,
                                         scale=SOFTCAP)
                else:
                    for half, (j, mix, q0, q1) in enumerate(pr):
                        o = half * 512
                        nc.scalar.activation(e[:, o + q0:o + q1],
                                             srcs[half][:, o + q0:o + q1],
                                             AF.Exp, scale=SOFTCAP)
                # bf16 accumulate: 2x DVE rate; per-partition rounding
                # errors average out in the final cross-partition sum
                for half, (j, mix, q0, q1) in enumerate(pr):
                    o = half * 512
                    if i == 0 and half == 0:
                        nc.vector.tensor_copy(zacc[:], e[:, 0:512])
                    else:
                        nc.vector.tensor_add(zacc[:, q0:q1], zacc[:, q0:q1],
                                             e[:, o + q0:o + q1])
                state[i] = e

            def mm_pair(i):
                pr = pairs[i]
                e = state.pop(i)
                for half, (j, mix, q0, q1) in enumerate(pr):
                    o = half * 512
                    first = i == 0 and half == 0
                    last = i == len(pairs) - 1 and half == 1
                    nc.tensor.matmul(olo[:, q0:q1],
                                     vt[:, j * 256:j * 256 + 128],
                                     e[:, o + q0:o + q1],
                                     start=first, stop=last)
                    nc.tensor.matmul(ohi[:, q0:q1],
                                     vt[:, j * 256 + 128:(j + 1) * 256],
                                     e[:, o + q0:o + q1],
                                     start=first, stop=last)

            def tail_unit():
                nc.vector.tensor_copy(alo[:, c0:c0 + 512], olo[:])
                nc.vector.tensor_copy(ahi[:, c0:c0 + 512], ohi[:])
                zps = ps_v.tile([1, 512], F32, tag="ps_v")
                nc.tensor.matmul(zps[:], ones[:], zacc[:],
                                 start=True, stop=True)
                zrow = zpool.tile([1, 512], F32, tag="z")
                nc.vector.tensor_copy(zrow[:], zps[:])
                # transpose Z [1,512] -> [128,4] with 4 tiny PE matmuls
                # (zrow slice as stationary, 1x1 ones as moving operand);
                # ~0.5us on-chip vs the ~5us DRAM round-trip it replaces
                zcps = ps_v.tile([128, 512], F32, tag="ps_v")
                for b in range(4):
                    nc.tensor.matmul(zcps[:, b:b + 1],
                                     zrow[0:1, b * 128:(b + 1) * 128],
                                     ones1[:], start=True, stop=True)
                nc.vector.reciprocal(rc[:, 4 * qb:4 * qb + 4],
                                     zcps[:, 0:4])

            units = [lambda: s_pair(0)]
            for i in range(1, len(pairs)):
                units.append(lambda i=i: (s_pair(i), mm_pair(i - 1)))
            units.append(lambda: (mm_pair(len(pairs) - 1), tail_unit()))
            return units

        # PE warmup: a few throwaway matmuls so HAM reaches 8/8 before
        # the first real accumulation
        scratch = wpool.tile([128, 512], BF16, tag="scratch")
        nc.vector.memset(scratch[:], 0.0)
        wps = ps_pair.tile([128, 1024], F32, tag="ps_pair")
        for _ in range(10):
            nc.tensor.matmul(wps[:, :512], scratch[:, :128], scratch[:],
                             start=True, stop=True)

        # output projection units (one per (tok-tile, feat-block)); the
        # 1/Z normalization is fused into the PSUM->SBUF copy. The five
        # feat-blocks of a tok-tile stage into one [128, 2304] bf16 tile
        # that leaves as a single full-rate DMA on the gpsimd queue.
        fbs = [(0, 512), (512, 512), (1024, 512), (1536, 512), (2048, 256)]
        osb_state = {}

        def proj_unit(t, fi, act_heavy):
            f0, fw = fbs[fi]
            ps = ps_v.tile([128, 512], F32, tag="ps_v")
            nc.tensor.matmul(ps[:, :fw], alo[:, t * 128:(t + 1) * 128],
                             wo[:, f0:f0 + fw], start=True, stop=False)
            nc.tensor.matmul(ps[:, :fw], ahi[:, t * 128:(t + 1) * 128],
                             wo[:, HID + f0:HID + f0 + fw],
                             start=False, stop=True)
            if fi == 0:
                osb = opool.tile([128, HID], BF16, tag="o", name=f"osb{t}")
                osb_state[t] = osb
            osb = osb_state[t]
            # early blocks run next to ACT-free phase-A work -> lean ACT;
            # tail blocks run next to ACT-bound B blocks -> lean DVE
            on_act = (fi % 2 == 0) if act_heavy else (fi == 4)
            if on_act:
                nc.scalar.activation(osb[:, f0:f0 + fw], ps[:, :fw], AF.Copy,
                                     scale=rc[:, t:t + 1])
            else:
                nc.vector.tensor_scalar_mul(osb[:, f0:f0 + fw], ps[:, :fw],
                                            rc[:, t:t + 1])
            # two pipelined half-DMAs per tile: the first goes out while
            # the last feat-blocks are still being projected
            if fi == 1:
                nc.gpsimd.dma_start(out_d[t * 128:(t + 1) * 128, :1024],
                                    osb[:, :1024])
            elif fi == len(fbs) - 1:
                osb = osb_state.pop(t)
                nc.gpsimd.dma_start(out_d[t * 128:(t + 1) * 128, 1024:],
                                    osb[:, 1024:])

        def phase_c(qb, tail=False):
            act_heavy = qb < 4 or qb == NT - 1
            t0 = 4 * qb
            if not tail:
                return [lambda t=t, fi=fi: proj_unit(t, fi, act_heavy)
                        for t in range(t0, t0 + 4)
                        for fi in range(len(fbs))]
            # tail ordering: two tiles advance together so PE matmuls stay
            # dense while copies trail on both ACT and DVE
            return [lambda t=t, fi=fi: proj_unit(t, fi, act_heavy)
                    for tp in (t0, t0 + 2)
                    for fi in range(len(fbs))
                    for t in (tp, tp + 1)]

        def weave(bunits, aunits):
            """Alternate A and B units, with each slot's independent A
            units emitted BEFORE the B unit so stalled B consumers never
            block independent A matmuls in the in-order PE queue."""
            out = []
            na, nb = len(aunits), len(bunits)
            ai = 0
            for bi, bu in enumerate(bunits):
                want = (bi + 1) * na // nb
                while ai < want:
                    out.append(aunits[ai])
                    ai += 1
                out.append(bu)
            out.extend(aunits[ai:])
            return out

        for u in phase_a(0):
            u()
        for u in phase_a(1):
            u()
        # process blocks 1..7 then 0: the last B block is the smallest so
        # the exposed tail after it is short. C phases lag their B block
        # and are partially deferred so the A-less tail blocks (B7, B0)
        # still have PE-heavy filler next to their ACT-bound softmax work.
        a7 = c5 = None
        for qb in [1, 2, 3, 4, 5, 6, 7, 0]:
            if qb == 0:
                fill = phase_c(7, tail=True)
            elif qb == 1:
                fill = phase_a(2)
            elif qb <= 5:
                fill = phase_a(qb + 1) + phase_c(qb - 1)
            elif qb == 6:
                a7 = phase_a(7)
                c5 = phase_c(5)
                fill = a7[:4] + c5[:10]
            elif qb == 7:
                fill = a7[4:] + c5[10:] + phase_c(6, tail=True)
            else:
                fill = phase_c(7, tail=True)
            alt = qb in (7, 0)
            with nc.named_scope(f"B{qb}"):
                for u in weave(phase_b(qb, alt=alt), fill):
                    u()
        with nc.named_scope("Ctail"):
            for u in phase_c(0, tail=True):
                u()


    split_multi_waits(nc)
    return nc


def _sbuf_image(mat, cols):
    """[R, cols] -> [128, (R//128)*cols] SBUF image (chunk-major rows)."""
    R = mat.shape[0]
    return np.ascontiguousarray(
        mat.reshape(R // 128, 128, cols).transpose(1, 0, 2).reshape(
            128, (R // 128) * cols))


def kernel(hidden_states, attention_mask, position_ids, Wqkv, Wo):
    bf16 = ml_dtypes.bfloat16
    hidden = np.asarray(hidden_states, np.float32)
    S = hidden.shape[1]
    NT = S // 512
    X = hidden[0]  # [S, HID]
    XT = np.ascontiguousarray(X.T).astype(bf16)  # [HID, S]
    # per-block SBUF images: [NT, 128, KC*512]
    xt_img = np.ascontiguousarray(
        XT.reshape(KC, 128, NT, 512).transpose(2, 1, 0, 3).reshape(
            NT, 128, KC * 512))

    pos = np.asarray(position_ids)[0].astype(np.float64)
    inv = 1.0 / (ROPE_THETA ** (np.arange(0, HD, 2, dtype=np.float64) / HD))
    freqs = inv[:, None] * pos[None, :]  # [128, S]
    cosT = np.cos(freqs).astype(np.float32)
    sinT = np.sin(freqs).astype(np.float32)

    plan, maskb = _classify_mask(attention_mask, S)
    nmix = maskb.shape[0]
    maskb_img = np.ascontiguousarray(
        maskb.transpose(1, 0, 2).reshape(128, nmix * 512))

    Wqkv = np.asarray(Wqkv, np.float32)
    Wo = np.asarray(Wo, np.float32)
    ones = np.ones((128, 1), bf16)

    in_maps = []
    for c in range(N_CORES):
        g = c // (NH // NKV)
        wq = Wqkv[c * HD:(c + 1) * HD] * SCALE  # exact: SCALE = 2**-4
        wk = Wqkv[NH * HD + g * HD: NH * HD + (g + 1) * HD]
        wv = Wqkv[(NH + NKV) * HD + g * HD: (NH + NKV) * HD + (g + 1) * HD]
        wqk = np.ascontiguousarray(
            np.concatenate([wq.T, wk.T], axis=1)).astype(bf16)  # [HID, 512]
        wvt = np.ascontiguousarray(wv.T).astype(bf16)           # [HID, 256]
        wot = np.ascontiguousarray(Wo[:, c * HD:(c + 1) * HD].T).astype(bf16)
        in_maps.append({
            "xt": xt_img,
            "wqk": _sbuf_image(wqk, 512),
            "wv": _sbuf_image(wvt, 256),
            "wo": _sbuf_image(wot, HID),
            "cost": cosT, "sint": sinT, "maskb": maskb_img,
        })

    nc = _build(S, plan, nmix)
    res = run_bass_kernel_spmd(nc, in_maps, list(range(N_CORES)),
                               trace=TRACE)
    out = res.results[0]["out"].astype(np.float64)
    for c in range(1, N_CORES):
        out += res.results[c]["out"].astype(np.float64)
    kernel.last_exec_time_ns = res.exec_time_ns
    kernel.last_results = res
    return out[None].astype(np.float32)


kernel.last_exec_time_ns = None
kernel.last_results = None



# revision 38
# speedup vs baseline: 1.0149x; 1.0116x over previous
"""Gemma2 fused attention (B=1, S=4096, HID=2304, NH=8, NKV=4, HD=256,
sliding window 2048, softcap 50) on 8 Trainium2 NeuronCores.

Sharding: one query head per core (its GQA kv head recomputed per core);
o_proj is sharded over the contraction dim, per-core partials are summed
on the host.

Per-core math (core c, head h=c, kv group g=c//2):
  qT,kT = (W @ X.T) in [head_dim, tok] layout, RoPE'd on device (cos/sin
  tables precomputed on host; attention scale folded into Wq exactly).
  v in [tok, head_dim] layout.
  S.T[k,q] = kT.T @ qT; u = tanh(S.T/50); E = exp(50*u + mask) in bf16
  (softcap bounds logits to +-50 so no max-subtraction is needed).
  Mask handled per 128(k) x 512(q) block: all-zero blocks skip the add,
  fully-masked blocks are skipped entirely, mixed blocks add mask*0.02
  from a host-packed block stack (data-driven, no pattern assumption).
  Z = ones.T @ E (PSUM row), attnT = (E @ v).T via lhsT=v chunks.
  out_partial[tok, 2304] = attnT.T @ WoT with 1/Z fused into the
  PSUM->SBUF copy. Host sums the 8 partials.
"""

import numpy as np
import ml_dtypes
from contextlib import ExitStack

import concourse.bass as bass
import concourse.tile as tile
import concourse.mybir as mybir
from concourse.bass_utils import run_bass_kernel_spmd
from concourse.vector_clock import ScopedClock

N_CORES = 8
HID = 2304
NH, NKV, HD = 8, 4, 256
SCALE = 256.0 ** -0.5
SOFTCAP = 50.0
ROPE_THETA = 10000.0
KC = HID // 128  # 18 contraction chunks for the projections

BF16 = mybir.dt.bfloat16
F32 = mybir.dt.float32
AF = mybir.ActivationFunctionType

TRACE = False  # test harness flips this to get NTFF exec time


class TC(tile.TileContext):
    """TileContext whose final drain splits sem waits one-per-instruction
    (this walrus rejects instructions carrying more than one wait)."""

    def _drain_and_barrier(self, tick_clock, wait_clock):
        probe = self.nc.sync.nop(nofuse=True, hint="drain_waits")
        wait_clock.add_sem_waits(
            probe.ins, ScopedClock({None: tick_clock.global_clock})
        )
        waits = list(probe.ins.sync_info.on_wait)
        probe.ins.sync_info.on_wait = waits[:1]
        rest = waits[1:]
        while rest:
            extra = self.nc.sync.nop(nofuse=True, hint="drain_waits")
            extra.ins.sync_info = mybir.SyncInfo(on_wait=rest[:1], on_update=[])
            rest = rest[1:]
        self.nc.sync.drain()
        self.nc.all_engine_barrier()
        popped = self.nc._tile_sem_poison_stack.pop()
        assert popped is self._sem_poison
        self.nc.clear_and_free_semaphores(list(self.sems.allocated().values()))
        self.nc.all_engine_barrier()


def split_multi_waits(nc):
    """Split multi-wait instructions: extras move onto same-engine NoOps
    inserted immediately before (engines execute in program order)."""
    ctr = 0
    for f in nc.m.functions:
        for b in f.blocks:
            insts = list(b.instructions)
            new = []
            changed = False
            for inst in insts:
                si = inst.sync_info
                if si is not None and len(si.on_wait) > 1:
                    waits = list(si.on_wait)
                    for w in waits[:-1]:
                        ctr += 1
                        nop = mybir.InstNoOp(
                            name=f"I-waitsplit-{ctr}",
                            engine=inst.engine,
                            debug=inst.debug,
                            sync_info=mybir.SyncInfo(on_wait=[w], on_update=[]),
                        )
                        new.append(nop)
                    inst.sync_info = mybir.SyncInfo(
                        on_wait=[waits[-1]], on_update=list(si.on_update)
                    )
                    changed = True
                new.append(inst)
            if changed:
                b.instructions = new
    return ctr


def _classify_mask(mask, S):
    """Per (k-chunk 128, q-block 512) block: 'skip' (fully masked),
    'clean' (all zero) or mixed (apply additively). Mixed blocks are
    deduplicated by content (causal/sliding-window masks repeat a handful
    of boundary patterns) so the whole stack stays resident in SBUF.
    Returns plan and the unique-block stack scaled by 1/SOFTCAP."""
    maskT = np.ascontiguousarray(np.asarray(mask, np.float32)[0, 0].T)  # [k, q]
    nj, nq = S // 128, S // 512
    blocks = maskT.reshape(nj, 128, nq, 512)
    mx = blocks.max(axis=(1, 3))
    mn = blocks.min(axis=(1, 3))
    skip = mx < -1e8
    clean = (mx == 0.0) & (mn == 0.0)
    plan = []
    mix_blocks = []
    uniq = {}

    def add_block(j, qb):
        """Returns (uniq_idx, q0, q1): dedup index + active column range
        (columns outside [q0,q1) are fully masked and skipped)."""
        raw = maskT[j * 128:(j + 1) * 128, qb * 512:(qb + 1) * 512]
        blk = raw.astype(ml_dtypes.bfloat16)
        key = blk.tobytes()
        if key not in uniq:
            uniq[key] = len(mix_blocks)
            mix_blocks.append(blk)
        act = np.where((raw > -1e8).any(axis=0))[0]
        q0, q1 = (int(act.min()), int(act.max()) + 1) if act.size else (0, 512)
        return uniq[key], q0, q1

    for qb in range(nq):
        row = []
        for j in range(nj):
            if skip[j, qb]:
                continue
            if clean[j, qb]:
                row.append((j, -1, 0, 512))
            else:
                row.append((j,) + add_block(j, qb))
        if not row:
            # fully-masked q-block (unreachable for causal masks): keep the
            # diagonal chunks so the PSUM accumulations are still defined
            for j in range(4 * qb, 4 * qb + 4):
                row.append((j,) + add_block(j, qb))
        # a full-width chunk must come first: it carries start=True for the
        # PSUM accumulation and the full-width zacc init
        row.sort(key=lambda e: 0 if e[3] - e[2] == 512 else 1)
        plan.append(row)
    if mix_blocks:
        maskb = np.stack(mix_blocks)  # [n, 128, 512] bf16
    else:
        maskb = np.zeros((1, 128, 512), ml_dtypes.bfloat16)
    return plan, maskb


def _build(S, plan, nmix):
    """Emit the SPMD program (identical for all cores; only data differs).

    All DRAM inputs are host-prepacked into exact SBUF images
    ([128 partitions, X] with long contiguous per-partition rows) so each
    tensor loads with a couple of large full-rate DMAs instead of dozens
    of 1KB-row transfers (each dma_start costs ~650ns of Sync-engine
    descriptor generation)."""
    NT = S // 512  # token/query 512-blocks
    nc = bass.Bass("TRN2", target_bir_lowering=False, debug=False,
                   num_devices=N_CORES)

    xt_d = nc.dram_tensor("xt", [NT, 128, KC * 512], BF16,
                          kind="ExternalInput")
    wqk_d = nc.dram_tensor("wqk", [128, KC * 512], BF16,
                           kind="ExternalInput")
    wv_d = nc.dram_tensor("wv", [128, KC * 256], BF16, kind="ExternalInput")
    wo_d = nc.dram_tensor("wo", [128, 2 * HID], BF16, kind="ExternalInput")
    cos_d = nc.dram_tensor("cost", [128, S], F32, kind="ExternalInput")
    sin_d = nc.dram_tensor("sint", [128, S], F32, kind="ExternalInput")
    maskb_d = nc.dram_tensor("maskb", [128, nmix * 512], BF16,
                             kind="ExternalInput")
    eye_d = nc.dram_tensor("eye", [128, 128], BF16, kind="ExternalInput")
    out_d = nc.dram_tensor("out", [S, HID], BF16, kind="ExternalOutput")

    with ExitStack() as ctx:
        tc = ctx.enter_context(TC(nc))
        P = lambda name, bufs, space="SBUF": ctx.enter_context(
            tc.tile_pool(name=name, bufs=bufs, space=space))

        wpool = P("w", 1)
        xpool = P("x", 2)
        cspool = P("cs", 2)
        qkpool = P("qk", 1)
        vpool = P("v", 1)
        tmppool = P("tmp", 4)
        upool = P("u", 3)
        epool = P("e", 4)
        apool = P("a", 1)
        zpool = P("z", 2)
        zapool = P("za", 2)
        opool = P("o", 3)
        rpool = P("r", 1)

        ps_qk = P("ps_qk", 1, "PSUM")
        ps_v = P("ps_v", 2, "PSUM")
        ps_pair = P("ps_pair", 1, "PSUM")
        ps_o = P("ps_o", 2, "PSUM")

        # --- resident weights / constants; wo + masks are DMA'd from
        # inside phase A block 0 AFTER the critical wqk/xt transfers so
        # they don't steal startup HBM bandwidth ---
        wqk = wpool.tile([128, KC * 512], BF16, tag="wqk")
        wv = wpool.tile([128, KC * 256], BF16, tag="wv")
        wo = wpool.tile([128, 2 * HID], BF16, tag="wo")
        masks = wpool.tile([128, nmix * 512], BF16, tag="masks")
        ones = wpool.tile([128, 1], BF16, tag="ones")
        nc.vector.memset(ones[:], 1.0)
        ones1 = wpool.tile([1, 1], F32, tag="ones1")
        nc.vector.memset(ones1[:], 1.0)
        eye = wpool.tile([128, 128], BF16, tag="eye")
        nc.sync.dma_start(eye[:], eye_d[:, :])

        # persistent activations (bf16, [128, S] each)
        qlo = qkpool.tile([128, S], BF16, tag="qlo")
        qhi = qkpool.tile([128, S], BF16, tag="qhi")
        klo = qkpool.tile([128, S], BF16, tag="klo")
        khi = qkpool.tile([128, S], BF16, tag="khi")
        vt = vpool.tile([128, (S // 128) * 256], BF16, tag="vt")
        alo = apool.tile([128, S], BF16, tag="alo")
        ahi = apool.tile([128, S], BF16, tag="ahi")
        rc = rpool.tile([128, S // 128], F32, tag="rc")

        qk_dst = [qlo, qhi, klo, khi]

        def phase_a(T):
            """Return emission units (closures) for QKV block T."""
            c0 = T * 512
            xt = xpool.tile([128, KC * 512], BF16, tag="xt")
            units = []

            def dma_unit():
                H = KC * 512 // 2
                if T == 0:
                    # growing interleaved pieces: the first (1-chunk) piece
                    # lands ~1.5us after issue so real matmuls start early;
                    # later pieces are big for full DMA rate. masks/wo/wv
                    # ride behind the critical transfers.
                    edges = [0, 1, 3, 6, 9, 12, 15, 18]
                    for g in range(len(edges) - 1):
                        a, b = edges[g] * 512, edges[g + 1] * 512
                        nc.sync.dma_start(wqk[:, a:b], wqk_d[:, a:b])
                        nc.sync.dma_start(xt[:, a:b], xt_d[T, :, a:b])
                    # non-critical loads ride on the SAME queue so they
                    # stay strictly behind the wqk/xt pieces above
                    nc.sync.dma_start(wv[:], wv_d[:, :])
                    nc.sync.dma_start(masks[:], maskb_d[:, :])
                    nc.sync.dma_start(wo[:], wo_d[:, :])
                else:
                    nc.sync.dma_start(xt[:, :H], xt_d[T, :, :H])
                    nc.sync.dma_start(xt[:, H:], xt_d[T, :, H:])
            units.append(dma_unit)

            cos = cspool.tile([128, 512], F32, tag="cos")
            sin = cspool.tile([128, 512], F32, tag="sin")

            def cs_unit():
                nc.sync.dma_start(cos[:], cos_d[:, c0:c0 + 512])
                nc.sync.dma_start(sin[:], sin_d[:, c0:c0 + 512])
            units.append(cs_unit)

            def qk_pair(p):
                qp = ps_qk.tile([128, 1024], F32, tag="ps_qk")
                for h in range(2):
                    ft = 2 * p + h
                    for kc in range(KC):
                        nc.tensor.matmul(
                            qp[:, h * 512:(h + 1) * 512],
                            wqk[:, kc * 512 + ft * 128:
                                kc * 512 + ft * 128 + 128],
                            xt[:, kc * 512:(kc + 1) * 512],
                            start=(kc == 0), stop=(kc == KC - 1))
                plo, phi = qp[:, 0:512], qp[:, 512:1024]
                dlo, dhi = qk_dst[2 * p], qk_dst[2 * p + 1]
                t1 = tmppool.tile([128, 512], F32, tag="tmp")
                nc.vector.tensor_mul(t1[:], phi, sin[:])
                t2 = tmppool.tile([128, 512], F32, tag="tmp")
                nc.vector.tensor_mul(t2[:], plo, cos[:])
                nc.vector.tensor_sub(dlo[:, c0:c0 + 512], t2[:], t1[:])
                t3 = tmppool.tile([128, 512], F32, tag="tmp")
                nc.vector.tensor_mul(t3[:], plo, sin[:])
                t4 = tmppool.tile([128, 512], F32, tag="tmp")
                nc.vector.tensor_mul(t4[:], phi, cos[:])
                nc.vector.tensor_add(dhi[:, c0:c0 + 512], t4[:], t3[:])

            def qk_chunk_major():
                # block 0 is paced by the weight/activation DMAs: keep 4
                # accumulations in flight (borrowing the idle B-phase pair
                # bank) so each arriving chunk feeds 4 matmuls
                qpA = ps_qk.tile([128, 1024], F32, tag="ps_qk")
                pspA = ps_pair.tile([128, 1024], F32, tag="ps_pair")
                psA = [qpA[:, 0:512], qpA[:, 512:1024],
                       pspA[:, 0:512], pspA[:, 512:1024]]
                for kc in range(KC):
                    for ft in range(4):
                        nc.tensor.matmul(
                            psA[ft],
                            wqk[:, kc * 512 + ft * 128: kc * 512 + ft * 128 + 128],
                            xt[:, kc * 512:(kc + 1) * 512],
                            start=(kc == 0), stop=(kc == KC - 1))
                for pair in range(2):
                    plo, phi = psA[2 * pair], psA[2 * pair + 1]
                    dlo, dhi = qk_dst[2 * pair], qk_dst[2 * pair + 1]
                    t1 = tmppool.tile([128, 512], F32, tag="tmp")
                    nc.vector.tensor_mul(t1[:], phi, sin[:])
                    t2 = tmppool.tile([128, 512], F32, tag="tmp")
                    nc.vector.tensor_mul(t2[:], plo, cos[:])
                    nc.vector.tensor_sub(dlo[:, c0:c0 + 512], t2[:], t1[:])
                    t3 = tmppool.tile([128, 512], F32, tag="tmp")
                    nc.vector.tensor_mul(t3[:], plo, sin[:])
                    t4 = tmppool.tile([128, 512], F32, tag="tmp")
                    nc.vector.tensor_mul(t4[:], phi, cos[:])
                    nc.vector.tensor_add(dhi[:, c0:c0 + 512], t4[:], t3[:])

            if T == 0:
                units.append(qk_chunk_major)
            else:
                for p in range(2):
                    units.append(lambda p=p: qk_pair(p))

            def v_unit(half):
                ps = ps_v.tile([128, 512], F32, tag="ps_v")
                for sub in range(2):
                    st = half * 2 + sub
                    o = ps[:, sub * 256:(sub + 1) * 256]
                    for kc in range(KC):
                        nc.tensor.matmul(
                            o,
                            xt[:, kc * 512 + st * 128: kc * 512 + st * 128 + 128],
                            wv[:, kc * 256:(kc + 1) * 256],
                            start=(kc == 0), stop=(kc == KC - 1))
                for sub in range(2):
                    st = half * 2 + sub
                    tok = T * 4 + st
                    # ACT is idle during phase A; keeping this off DVE also
                    # stops the v evacuation queueing behind RoPE ops
                    nc.scalar.copy(vt[:, tok * 256:(tok + 1) * 256],
                                   ps[:, sub * 256:(sub + 1) * 256])

            for half in range(2):
                units.append(lambda half=half: v_unit(half))
            return units

        def phase_b(qb, alt=False):
            """Return emission units for attention q-block qb, one per
            k-chunk PAIR: two chunks share a 2-bank PSUM tile so full-width
            pairs get a single wide tanh/exp (halves the ACT op count).
            The S matmuls of pair i lead the E-consumers of pair i-1 so the
            ACT chain has a full PE iteration of slack."""
            c0 = qb * 512
            zacc = zapool.tile([128, 512], BF16, tag="za")
            olo = ps_o.tile([128, 512], F32, tag="ps_o")
            ohi = ps_o.tile([128, 512], F32, tag="ps_o")
            row = plan[qb]
            assert len(row) % 2 == 0
            pairs = [(row[2 * i], row[2 * i + 1]) for i in range(len(row) // 2)]
            state = {}

            def s_pair(i):
                pr = pairs[i]
                # in blocks with no woven A work (tail), alternate between
                # the two 2-bank pools for 2-deep pair pipelining
                pool = ps_qk if (alt and i % 2 == 1) else ps_pair
                psp = pool.tile([128, 1024], F32, tag=pool.name, name="psp")
                for half, (j, mix, q0, q1) in enumerate(pr):
                    o = half * 512
                    nc.tensor.matmul(psp[:, o + q0:o + q1],
                                     klo[:, j * 128:(j + 1) * 128],
                                     qlo[:, c0 + q0:c0 + q1],
                                     start=True, stop=False)
                    nc.tensor.matmul(psp[:, o + q0:o + q1],
                                     khi[:, j * 128:(j + 1) * 128],
                                     qhi[:, c0 + q0:c0 + q1],
                                     start=False, stop=(mix < 0))
                    if mix >= 0:
                        # fold the additive mask into the S accumulation on
                        # the PE (identity x mask block): masked entries
                        # become ~-1e9 -> tanh -1 -> exp(-50) -> 0 in bf16,
                        # keeping the whole softcap chain on ACT only
                        nc.tensor.matmul(psp[:, o + q0:o + q1], eye[:],
                                         masks[:, mix * 512 + q0:
                                               mix * 512 + q1],
                                         start=False, stop=True)
                u = upool.tile([128, 1024], F32, tag="u")
                e = epool.tile([128, 1024], BF16, tag="e")
                both_full = all(q1 - q0 == 512 for _, _, q0, q1 in pr)
                if both_full:
                    nc.scalar.activation(u[:], psp[:], AF.Tanh,
                                         scale=1.0 / SOFTCAP)
                    nc.scalar.activation(e[:], u[:], AF.Exp, scale=SOFTCAP)
                else:
                    for half, (j, mix, q0, q1) in enumerate(pr):
                        o = half * 512
                        nc.scalar.activation(u[:, o + q0:o + q1],
                                             psp[:, o + q0:o + q1], AF.Tanh,
                                             scale=1.0 / SOFTCAP)
                        nc.scalar.activation(e[:, o + q0:o + q1],
                                             u[:, o + q0:o + q1],
                                             AF.Exp, scale=SOFTCAP)
                # bf16 accumulate: 2x DVE rate; per-partition rounding
                # errors average out in the final cross-partition sum
                for half, (j, mix, q0, q1) in enumerate(pr):
                    o = half * 512
                    if i == 0 and half == 0:
                        nc.vector.tensor_copy(zacc[:], e[:, 0:512])
                    else:
                        nc.vector.tensor_add(zacc[:, q0:q1], zacc[:, q0:q1],
                                             e[:, o + q0:o + q1])
                state[i] = e

            def mm_pair(i):
                pr = pairs[i]
                e = state.pop(i)
                for half, (j, mix, q0, q1) in enumerate(pr):
                    o = half * 512
                    first = i == 0 and half == 0
                    last = i == len(pairs) - 1 and half == 1
                    nc.tensor.matmul(olo[:, q0:q1],
                                     vt[:, j * 256:j * 256 + 128],
                                     e[:, o + q0:o + q1],
                                     start=first, stop=last)
                    nc.tensor.matmul(ohi[:, q0:q1],
                                     vt[:, j * 256 + 128:(j + 1) * 256],
                                     e[:, o + q0:o + q1],
                                     start=first, stop=last)

            def tail_unit():
                nc.vector.tensor_copy(alo[:, c0:c0 + 512], olo[:])
                nc.vector.tensor_copy(ahi[:, c0:c0 + 512], ohi[:])
                zps = ps_v.tile([1, 512], F32, tag="ps_v")
                nc.tensor.matmul(zps[:], ones[:], zacc[:],
                                 start=True, stop=True)
                zrow = zpool.tile([1, 512], F32, tag="z")
                nc.vector.tensor_copy(zrow[:], zps[:])
                # transpose Z [1,512] -> [128,4] with 4 tiny PE matmuls
                # (zrow slice as stationary, 1x1 ones as moving operand);
                # ~0.5us on-chip vs the ~5us DRAM round-trip it replaces
                zcps = ps_v.tile([128, 512], F32, tag="ps_v")
                for b in range(4):
                    nc.tensor.matmul(zcps[:, b:b + 1],
                                     zrow[0:1, b * 128:(b + 1) * 128],
                                     ones1[:], start=True, stop=True)
                nc.vector.reciprocal(rc[:, 4 * qb:4 * qb + 4],
                                     zcps[:, 0:4])

            units = [lambda: s_pair(0)]
            for i in range(1, len(pairs)):
                units.append(lambda i=i: (s_pair(i), mm_pair(i - 1)))
            units.append(lambda: (mm_pair(len(pairs) - 1), tail_unit()))
            return units

        # PE warmup: a few throwaway matmuls so HAM reaches 8/8 before
        # the first real accumulation
        scratch = wpool.tile([128, 512], BF16, tag="scratch")
        nc.vector.memset(scratch[:], 0.0)
        wps = ps_pair.tile([128, 1024], F32, tag="ps_pair")
        for _ in range(10):
            nc.tensor.matmul(wps[:, :512], scratch[:, :128], scratch[:],
                             start=True, stop=True)

        # output projection units (one per (tok-tile, feat-block)); the
        # 1/Z normalization is fused into the PSUM->SBUF copy. The five
        # feat-blocks of a tok-tile stage into one [128, 2304] bf16 tile
        # that leaves as a single full-rate DMA on the gpsimd queue.
        fbs = [(0, 512), (512, 512), (1024, 512), (1536, 512), (2048, 256)]
        osb_state = {}

        def proj_unit(t, fi, act_heavy):
            f0, fw = fbs[fi]
            ps = ps_v.tile([128, 512], F32, tag="ps_v")
            nc.tensor.matmul(ps[:, :fw], alo[:, t * 128:(t + 1) * 128],
                             wo[:, f0:f0 + fw], start=True, stop=False)
            nc.tensor.matmul(ps[:, :fw], ahi[:, t * 128:(t + 1) * 128],
                             wo[:, HID + f0:HID + f0 + fw],
                             start=False, stop=True)
            if fi == 0:
                osb = opool.tile([128, HID], BF16, tag="o", name=f"osb{t}")
                osb_state[t] = osb
            osb = osb_state[t]
            # early blocks run next to ACT-free phase-A work -> lean ACT;
            # tail blocks run next to ACT-bound B blocks -> lean DVE
            on_act = (fi % 2 == 0) if act_heavy else (fi == 4)
            if on_act:
                nc.scalar.activation(osb[:, f0:f0 + fw], ps[:, :fw], AF.Copy,
                                     scale=rc[:, t:t + 1])
            else:
                nc.vector.tensor_scalar_mul(osb[:, f0:f0 + fw], ps[:, :fw],
                                            rc[:, t:t + 1])
            # two pipelined half-DMAs per tile: the first goes out while
            # the last feat-blocks are still being projected
            if fi == 1:
                nc.gpsimd.dma_start(out_d[t * 128:(t + 1) * 128, :1024],
                                    osb[:, :1024])
            elif fi == len(fbs) - 1:
                osb = osb_state.pop(t)
                nc.gpsimd.dma_start(out_d[t * 128:(t + 1) * 128, 1024:],
                                    osb[:, 1024:])

        def phase_c(qb, tail=False):
            act_heavy = qb < 4 or qb == NT - 1
            t0 = 4 * qb
            if not tail:
                return [lambda t=t, fi=fi: proj_unit(t, fi, act_heavy)
                        for t in range(t0, t0 + 4)
                        for fi in range(len(fbs))]
            # tail ordering: two tiles advance together so PE matmuls stay
            # dense while copies trail on both ACT and DVE
            return [lambda t=t, fi=fi: proj_unit(t, fi, act_heavy)
                    for tp in (t0, t0 + 2)
                    for fi in range(len(fbs))
                    for t in (tp, tp + 1)]

        def weave(bunits, aunits):
            """Alternate A and B units, with each slot's independent A
            units emitted BEFORE the B unit so stalled B consumers never
            block independent A matmuls in the in-order PE queue."""
            out = []
            na, nb = len(aunits), len(bunits)
            ai = 0
            for bi, bu in enumerate(bunits):
                want = (bi + 1) * na // nb
                while ai < want:
                    out.append(aunits[ai])
                    ai += 1
                out.append(bu)
            out.extend(aunits[ai:])
            return out

        for u in phase_a(0):
            u()
        for T in range(NT):
            alt = T == NT - 1
            fill = phase_a(T + 1) if T + 1 < NT else []
            if T >= 1:
                fill = fill + phase_c(T - 1, tail=alt)
            with nc.named_scope(f"B{T}"):
                for u in weave(phase_b(T, alt=alt), fill):
                    u()
        with nc.named_scope("Ctail"):
            for u in phase_c(NT - 1, tail=True):
                u()

    split_multi_waits(nc)
    return nc


def _sbuf_image(mat, cols):
    """[R, cols] -> [128, (R//128)*cols] SBUF image (chunk-major rows)."""
    R = mat.shape[0]
    return np.ascontiguousarray(
        mat.reshape(R // 128, 128, cols).transpose(1, 0, 2).reshape(
            128, (R // 128) * cols))


def kernel(hidden_states, attention_mask, position_ids, Wqkv, Wo):
    bf16 = ml_dtypes.bfloat16
    hidden = np.asarray(hidden_states, np.float32)
    S = hidden.shape[1]
    NT = S // 512
    X = hidden[0]  # [S, HID]
    XT = np.ascontiguousarray(X.T).astype(bf16)  # [HID, S]
    # per-block SBUF images: [NT, 128, KC*512]
    xt_img = np.ascontiguousarray(
        XT.reshape(KC, 128, NT, 512).transpose(2, 1, 0, 3).reshape(
            NT, 128, KC * 512))

    pos = np.asarray(position_ids)[0].astype(np.float64)
    inv = 1.0 / (ROPE_THETA ** (np.arange(0, HD, 2, dtype=np.float64) / HD))
    freqs = inv[:, None] * pos[None, :]  # [128, S]
    cosT = np.cos(freqs).astype(np.float32)
    sinT = np.sin(freqs).astype(np.float32)

    plan, maskb = _classify_mask(attention_mask, S)
    nmix = maskb.shape[0]
    maskb_img = np.ascontiguousarray(
        maskb.transpose(1, 0, 2).reshape(128, nmix * 512))

    Wqkv = np.asarray(Wqkv, np.float32)
    Wo = np.asarray(Wo, np.float32)
    eye128 = np.eye(128, dtype=bf16)

    in_maps = []
    for c in range(N_CORES):
        g = c // (NH // NKV)
        wq = Wqkv[c * HD:(c + 1) * HD] * SCALE  # exact: SCALE = 2**-4
        wk = Wqkv[NH * HD + g * HD: NH * HD + (g + 1) * HD]
        wv = Wqkv[(NH + NKV) * HD + g * HD: (NH + NKV) * HD + (g + 1) * HD]
        wqk = np.ascontiguousarray(
            np.concatenate([wq.T, wk.T], axis=1)).astype(bf16)  # [HID, 512]
        wvt = np.ascontiguousarray(wv.T).astype(bf16)           # [HID, 256]
        wot = np.ascontiguousarray(Wo[:, c * HD:(c + 1) * HD].T).astype(bf16)
        in_maps.append({
            "xt": xt_img,
            "wqk": _sbuf_image(wqk, 512),
            "wv": _sbuf_image(wvt, 256),
            "wo": _sbuf_image(wot, HID),
            "cost": cosT, "sint": sinT, "maskb": maskb_img,
            "eye": eye128,
        })

    nc = _build(S, plan, nmix)
    res = run_bass_kernel_spmd(nc, in_maps, list(range(N_CORES)),
                               trace=TRACE)
    out = res.results[0]["out"].astype(np.float64)
    for c in range(1, N_CORES):
        out += res.results[c]["out"].astype(np.float64)
    kernel.last_exec_time_ns = res.exec_time_ns
    kernel.last_results = res
    return out[None].astype(np.float32)


kernel.last_exec_time_ns = None
kernel.last_results = None

